# revision 10
# baseline (speedup 1.0000x reference)
"""Trainium2 Bass kernel: 2-layer PixelLSTM (B=512, T=365, D=16, S=32, H=256).

Data-parallel over the pixel/batch dim across 8 NeuronCores (64 pixels/core).
Per core, one fused recurrence loop computes layer0 step t and layer1 step t-1
together on the 128 SBUF partitions (rows 0:64 = L0 batch, 64:128 = L1 batch):

  - gate pre-activations accumulate in PSUM via float32r matmuls; the input
    contribution (x_t / h0_t) and the biases ride in the contraction stack
    (lhsT k-tiles: hT halves + [x_t;1] / ones row), so there is no separate
    "+xg" or "+bias" elementwise op.
  - layer0 and layer1 matmuls target disjoint 64-column groups of the PE
    array (tile_position), so their streams overlap.
  - the next step's stationary operand h^T is produced by two PE transposes
    of the combined h tile and one PSUM->SBUF copy.
  - the output projection (H->1) accumulates per step into one persistent
    PSUM bank column; bias bo is added once at the end.
"""

import sys
import os
import time

sys.path.insert(0, "/opt/trn_rl_repo")

import numpy as np
from contextlib import ExitStack

import concourse.bass as bass
import concourse.bacc as bacc
import concourse.tile as tile
from concourse import mybir
from concourse.masks import make_identity

B, T, D, S, H = 512, 365, 16, 32, 256
G4 = 4 * H
NCORES = 8
BL = B // NCORES  # 64 pixels per core

F32 = mybir.dt.float32
F32R = mybir.dt.float32r
BF16 = mybir.dt.bfloat16
AF = mybir.ActivationFunctionType


def _r(ap):
    """View an fp32 AP as float32r so the PE streams 1 cycle/row (N>=256)."""
    return ap.bitcast(F32R)


def _build(t_steps=T):
    nc = bacc.Bacc("TRN2", target_bir_lowering=False, debug=False)
    dram = {}
    for name, shape in [
        ("x", [BL, t_steps, D]),
        ("x_static", [BL, S]),
        ("Wih0", [G4, D]),
        ("Whh0", [G4, H]),
        ("bih0", [G4]),
        ("bhh0", [G4]),
        ("Wih1", [G4, H]),
        ("Whh1", [G4, H]),
        ("bih1", [G4]),
        ("bhh1", [G4]),
        ("Ws", [H, S]),
        ("bs", [H]),
        ("Wo", [1, H]),
        ("bo", [1]),
    ]:
        dram[name] = nc.dram_tensor(name, shape, F32, kind="ExternalInput")
    y = nc.dram_tensor("y", [BL, t_steps], F32, kind="ExternalOutput")

    with tile.TileContext(nc) as tc, ExitStack() as ctx:
        const = ctx.enter_context(tc.tile_pool(name="const", bufs=1))
        stg = ctx.enter_context(tc.tile_pool(name="stg", bufs=2))
        sb = ctx.enter_context(tc.tile_pool(name="sb", bufs=3))
        st = ctx.enter_context(tc.tile_pool(name="st", bufs=1))
        psg = ctx.enter_context(tc.tile_pool(name="psg", bufs=2, space="PSUM"))
        psh = ctx.enter_context(tc.tile_pool(name="psh", bufs=2, space="PSUM"))
        pso = ctx.enter_context(tc.tile_pool(name="pso", bufs=1, space="PSUM"))

        dma = nc.sync.dma_start
        cvt = nc.vector.tensor_copy

        # ---- weights: strided-DMA transposed loads into fp32 staging, then
        # one DVE convert each into the bf16 tiles the PE consumes ----
        def load_bf16(name_bf, src_ap, p, f):
            t_bf = const.tile([p, f], BF16, name=name_bf)
            t_st = stg.tile([128, G4], F32, tag="wstage", name=name_bf + "_st")
            dma(out=t_st[0:p, 0:f], in_=src_ap)
            cvt(t_bf, t_st[0:p, 0:f])
            return t_bf

        WhhT0 = dram["Whh0"].ap().rearrange("g h -> h g")  # [256, 1024]
        W0a = load_bf16("W0a", WhhT0[0:128, :], 128, G4)
        W0b = load_bf16("W0b", WhhT0[128:256, :], 128, G4)
        WhhT1 = dram["Whh1"].ap().rearrange("g h -> h g")
        W1a = load_bf16("W1a", WhhT1[0:128, :], 128, G4)
        W1b = load_bf16("W1b", WhhT1[128:256, :], 128, G4)
        WihT1 = dram["Wih1"].ap().rearrange("g h -> h g")
        W1c = load_bf16("W1c", WihT1[0:128, :], 128, G4)
        W1d = load_bf16("W1d", WihT1[128:256, :], 128, G4)

        # [Wih0^T ; bias0] : K = D+1
        W0c = const.tile([D + 1, G4], BF16)
        Wih0T = dram["Wih0"].ap().rearrange("g d -> d g")
        w0c_st = stg.tile([D, G4], F32, tag="wstage")
        dma(out=w0c_st, in_=Wih0T)
        cvt(W0c[0:D, :], w0c_st)
        b0a = stg.tile([1, G4], F32, tag="brow")
        dma(out=b0a, in_=dram["bih0"].ap().rearrange("(a g) -> a g", a=1))
        b0b = stg.tile([1, G4], F32, tag="brow")
        dma(out=b0b, in_=dram["bhh0"].ap().rearrange("(a g) -> a g", a=1))
        nc.vector.tensor_add(b0a, b0a, b0b)
        b0bf = stg.tile([1, G4], BF16, tag="brbf")
        cvt(b0bf, b0a)
        dma(out=W0c[D : D + 1, :], in_=b0bf)  # SBUF->SBUF partition move

        W1e = const.tile([1, G4], BF16)  # bias1 row
        b1a = stg.tile([1, G4], F32, tag="brow")
        dma(out=b1a, in_=dram["bih1"].ap().rearrange("(a g) -> a g", a=1))
        b1b = stg.tile([1, G4], F32, tag="brow")
        dma(out=b1b, in_=dram["bhh1"].ap().rearrange("(a g) -> a g", a=1))
        nc.vector.tensor_add(b1a, b1a, b1b)
        cvt(W1e, b1a)

        WoT = dram["Wo"].ap().rearrange("o h -> h o")  # [256, 1]
        WoTa = load_bf16("WoTa", WoT[0:128, :], 128, 1)
        WoTb = load_bf16("WoTb", WoT[128:256, :], 128, 1)
        bo_bc = const.tile([BL, 1], F32)
        bo_ap = dram["bo"].ap()
        dma(
            out=bo_bc,
            in_=bass.AP(tensor=bo_ap.tensor, offset=bo_ap.offset, ap=[[0, BL], [1, 1]]),
        )

        ident = const.tile([128, 128], BF16)
        make_identity(nc, ident)
        ones_row = const.tile([1, 2048], F32)
        nc.vector.memset(ones_row, 1.0)
        ones64 = const.tile([1, BL], BF16)
        cvt(ones64, ones_row[0:1, 0:BL])

        # ---- x transposed: [D+1, T*BL] bf16; row D is the all-ones row ----
        xaugT = const.tile([D + 1, t_steps * BL], BF16)
        xv = xaugT.rearrange("d (t b) -> d t b", b=BL)
        xT_src = dram["x"].ap().rearrange("b t d -> d t b")
        NCH = 8
        tch = (t_steps + NCH - 1) // NCH
        for chi in range(NCH):
            c0, c1 = chi * tch, min((chi + 1) * tch, t_steps)
            if c0 >= c1:
                break
            x_st = stg.tile([D + 1, tch * BL], F32, tag="xstage", name=f"x_st{chi}")
            xsv = x_st.rearrange("d (t b) -> d t b", b=BL)
            for d in range(D):
                dma(out=xsv[d : d + 1, 0 : c1 - c0, :], in_=xT_src[d : d + 1, c0:c1, :])
            for tt in range(c0, c1, 32):
                te = min(tt + 32, c1)
                dma(
                    out=x_st[D : D + 1, (tt - c0) * BL : (te - c0) * BL],
                    in_=ones_row[0:1, 0 : (te - tt) * BL],
                )
            cvt(
                xaugT[:, c0 * BL : c1 * BL],
                x_st[:, 0 : (c1 - c0) * BL],
            )

        # ---- x_static^T (aug with ones) and [Ws^T ; bs] ----
        xsT = const.tile([S + 1, BL], BF16)
        xst_st = stg.tile([S + 1, BL], F32, tag="brow")
        dma(out=xst_st[0:S, :], in_=dram["x_static"].ap().rearrange("b s -> s b"))
        dma(out=xst_st[S : S + 1, :], in_=ones_row[0:1, 0:BL])
        cvt(xsT, xst_st)
        WsT = const.tile([S + 1, H], BF16)
        wst_st = stg.tile([S + 1, H], F32, tag="brow")
        dma(out=wst_st[0:S, :], in_=dram["Ws"].ap().rearrange("h s -> s h"))
        dma(out=wst_st[S : S + 1, :], in_=dram["bs"].ap().rearrange("(a h) -> a h", a=1))
        cvt(WsT, wst_st)

        # ---- init: h0 = c0 = x_static @ Ws^T + bs, in both partition halves ----
        mm = nc.tensor.matmul
        hin_ps = psg.tile([128, H], F32, tag="g")
        mm(hin_ps[0:BL, :], lhsT=xsT, rhs=WsT, start=True, stop=True)
        mm(
            hin_ps[BL:128, :],
            lhsT=xsT,
            rhs=WsT,
            start=True,
            stop=True,
            tile_position=(0, BL),
        )
        c = st.tile([128, H], F32)
        nc.vector.tensor_copy(c, hin_ps)
        h_init = sb.tile([128, H], BF16, tag="h")
        nc.vector.tensor_copy(h_init, hin_ps)
        hT_ps0 = psh.tile([128, 2 * 128], BF16, tag="hT")
        nc.tensor.transpose(hT_ps0[:, 0:128], h_init[:, 0:128], ident)
        nc.tensor.transpose(hT_ps0[:, 128:256], h_init[:, 128:256], ident)
        hT_prev = sb.tile([128, 256], BF16, tag="hTs")
        nc.vector.tensor_copy(hT_prev, hT_ps0)

        out_ps = pso.tile([BL, t_steps], F32)

        act = nc.scalar.activation
        NH = 512  # matmul moving-operand half (PSUM bank width in fp32)

        for i in range(t_steps + 1):
            do0 = i < t_steps  # layer0 step i
            do1 = i >= 1  # layer1 step i-1
            lo = 0 if do0 else BL
            hi = 128 if do1 else BL
            sl = slice(lo, hi)

            gates = psg.tile([128, G4], F32, tag="g")
            for nh in range(2):
                ns = slice(nh * NH, (nh + 1) * NH)
                if do0:
                    g0 = gates[0:BL, ns]
                    mm(g0, lhsT=hT_prev[:, 0:BL], rhs=W0a[:, ns], start=True, stop=False)
                    mm(g0, lhsT=hT_prev[:, 128 : 128 + BL], rhs=W0b[:, ns], start=False, stop=False)
                    mm(g0, lhsT=xv[:, i, :], rhs=W0c[:, ns], start=False, stop=True)
                if do1:
                    g1 = gates[BL:128, ns]
                    tp = (0, BL)
                    mm(g1, lhsT=hT_prev[:, BL:128], rhs=W1a[:, ns], start=True, stop=False, tile_position=tp)
                    mm(g1, lhsT=hT_prev[:, 128 + BL : 256], rhs=W1b[:, ns], start=False, stop=False, tile_position=tp)
                    mm(g1, lhsT=hT_prev[:, 0:BL], rhs=W1c[:, ns], start=False, stop=False, tile_position=tp)
                    mm(g1, lhsT=hT_prev[:, 128 : 128 + BL], rhs=W1d[:, ns], start=False, stop=False, tile_position=tp)
                    mm(g1, lhsT=ones64, rhs=W1e[:, ns], start=False, stop=True, tile_position=tp)

            # gate order i,f,g,o: sigmoid(i,f) | tanh(g) | sigmoid(o)
            sif = sb.tile([128, 2 * H], F32, tag="sif")
            gg = sb.tile([128, H], F32, tag="gg")
            so = sb.tile([128, H], F32, tag="so")
            act(sif[sl], gates[sl, 0 : 2 * H], AF.Sigmoid)
            act(gg[sl], gates[sl, 2 * H : 3 * H], AF.Tanh)
            act(so[sl], gates[sl, 3 * H : 4 * H], AF.Sigmoid)

            t1 = sb.tile([128, H], F32, tag="t1")
            t2 = sb.tile([128, H], F32, tag="t2")
            nc.vector.tensor_mul(t1[sl], sif[sl, H : 2 * H], c[sl])  # f*c
            nc.vector.tensor_mul(t2[sl], sif[sl, 0:H], gg[sl])  # i*g
            nc.vector.tensor_add(c[sl], t1[sl], t2[sl])
            tc_ = sb.tile([128, H], F32, tag="tc")
            act(tc_[sl], c[sl], AF.Tanh)
            h = sb.tile([128, H], BF16, tag="h")
            nc.vector.tensor_mul(h[sl], so[sl], tc_[sl])
            if i == 0:
                # layer1's state before its step 0 is the shared static init
                nc.vector.tensor_copy(h[BL:128], hin_ps[BL:128])

            hT_ps = psh.tile([128, 256], BF16, tag="hT")
            nc.tensor.transpose(hT_ps[:, 0:128], h[:, 0:128], ident)
            nc.tensor.transpose(hT_ps[:, 128:256], h[:, 128:256], ident)
            hT_cur = sb.tile([128, 256], BF16, tag="hTs")
            nc.vector.tensor_copy(hT_cur, hT_ps)

            if do1:
                o_col = out_ps[0:BL, i - 1 : i]
                mm(o_col, lhsT=hT_cur[:, BL:128], rhs=WoTa, start=True, stop=False)
                mm(o_col, lhsT=hT_cur[:, 128 + BL : 256], rhs=WoTb, start=False, stop=True)

            hT_prev = hT_cur

        yt = sb.tile([BL, t_steps], F32, tag="yt")
        act(yt, out_ps, AF.Identity, bias=bo_bc, scale=1.0)
        dma(out=y.ap(), in_=yt)

    nc.compile()
    return nc


# ---------------------------------------------------------------------------
# SPMD runner with a cached jitted executable (so repeated calls don't re-jit)
# ---------------------------------------------------------------------------
_RUNNER = None


def _make_runner(nc, n_cores):
    import jax
    from jax.sharding import Mesh, PartitionSpec
    from jax.experimental.shard_map import shard_map
    from concourse.bass2jax import (
        _bass_exec_p,
        install_neuronx_cc_hook,
        partition_id_tensor,
    )

    install_neuronx_cc_hook()
    partition_name = nc.partition_id_tensor.name if nc.partition_id_tensor else None
    in_names, out_names, out_avals, zero_outs = [], [], [], []
    for alloc in nc.m.functions[0].allocations:
        if not isinstance(alloc, mybir.MemoryLocationSet):
            continue
        name = alloc.memorylocations[0].name
        if alloc.kind == "ExternalInput":
            if name != partition_name:
                in_names.append(name)
        elif alloc.kind == "ExternalOutput":
            shape = tuple(alloc.tensor_shape)
            dtp = mybir.dt.np(alloc.dtype)
            out_names.append(name)
            out_avals.append(jax.core.ShapedArray(shape, dtp))
            zero_outs.append(np.zeros(shape, dtp))
    n_params = len(in_names)
    n_outs = len(out_avals)
    all_in_names = list(in_names) + list(out_names)
    if partition_name is not None:
        all_in_names.append(partition_name)
    donate = tuple(range(n_params, n_params + n_outs))

    def _body(*args):
        operands = list(args)
        if partition_name is not None:
            operands.append(partition_id_tensor())
        outs = _bass_exec_p.bind(
            *operands,
            out_avals=tuple(out_avals),
            in_names=tuple(all_in_names),
            out_names=tuple(out_names),
            lowering_input_output_aliases=(),
            sim_require_finite=True,
            sim_require_nnan=True,
            nc=nc,
        )
        return tuple(outs)

    devices = jax.devices()[:n_cores]
    mesh = Mesh(np.asarray(devices), ("core",))
    in_specs = (PartitionSpec("core"),) * (n_params + n_outs)
    out_specs = (PartitionSpec("core"),) * n_outs
    sharded = jax.jit(
        shard_map(_body, mesh=mesh, in_specs=in_specs, out_specs=out_specs, check_rep=False),
        donate_argnums=donate,
        keep_unused=True,
    )

    def run(in_maps):
        concat_in = [
            np.concatenate([np.asarray(in_maps[c][name]) for c in range(n_cores)], axis=0)
            for name in in_names
        ]
        concat_zeros = [
            np.zeros((n_cores * z.shape[0], *z.shape[1:]), z.dtype) for z in zero_outs
        ]
        out_arrs = sharded(*concat_in, *concat_zeros)
        out_arrs = [np.asarray(a) for a in out_arrs]
        return [
            {
                name: out_arrs[i].reshape(n_cores, *out_avals[i].shape)[cc]
                for i, name in enumerate(out_names)
            }
            for cc in range(n_cores)
        ]

    return run


def _get_runner():
    global _RUNNER
    if _RUNNER is None:
        nc = _build(T)
        _RUNNER = _make_runner(nc, NCORES)
    return _RUNNER


def _make_in_maps(inputs):
    inputs = {k: np.ascontiguousarray(np.asarray(v), dtype=np.float32) for k, v in inputs.items()}
    weight_names = [
        "Wih0", "Whh0", "bih0", "bhh0", "Wih1", "Whh1", "bih1", "bhh1",
        "Ws", "bs", "Wo", "bo",
    ]
    in_maps = []
    for cidx in range(NCORES):
        bsl = slice(cidx * BL, (cidx + 1) * BL)
        m = {"x": inputs["x"][bsl], "x_static": inputs["x_static"][bsl]}
        for w in weight_names:
            m[w] = inputs[w]
        in_maps.append(m)
    return in_maps


def kernel(**inputs):
    run = _get_runner()
    results = _make_in_maps(inputs)
    outs = run(results)
    return np.concatenate([outs[c]["y"] for c in range(NCORES)], axis=0)
